# revision 22
# baseline (speedup 1.0000x reference)
"""Trainium2 Bass kernel for nn_CombinedAMLModel (dense_mlp, 8 NeuronCores).

Sharding: tensor-parallel over the gene axis (20000 genes -> 2500 per core).

Per core:
  Phase A  - per-(tech,gene) 1->4->1 MLPs plus the per-gene tech combinor,
             computed as 12 relu-affine passes (genes on partitions, per-
             partition scale/bias on ACT/DVE), accumulated into PSUM with
             diagonal fp32r matmuls whose diagonals carry W2[t,g,h]*Wc[g,t].
             The constant term (sum_t b2*Wc + bc) is added during the
             PSUM->SBUF copy. Produces z[g_local, s] (2500 x 1024).
  Phase B  - out1T[n, s] += CW0T[g, n].T @ z[g, s]  (fp32r, K=2500 local
             genes, n=2000), written to DRAM as this core's partial.
  Phase C  - AllReduce of the (2000, 1024) partials across 8 cores.
  Phase D  - tail MLP 2000->200->20->1, replicated on every core, computed
             entirely in transposed orientation (layer outputs on partitions,
             samples on the free axis) so no transposes are needed anywhere.

All matmuls run in float32r (full-rate fp32 matmul, ~1e-4 relative error).
"""
import os
import sys

sys.path.insert(0, "/opt/trn_rl_repo")

import ml_dtypes
import numpy as np
from contextlib import ExitStack

import concourse.bass as bass
import concourse.tile as tile
from concourse import bacc, mybir
from concourse.bass_utils import run_bass_kernel_spmd

T, S, G, H = 3, 1024, 20000, 4
NCORES = 8
GL = G // NCORES            # genes per core
PT = 125                    # gene-tile partition size
NGT = GL // PT              # gene tiles per core
NK = T * H                  # local relu-affine passes
N1, N2, N3 = 2000, 200, 20
PN = 125                    # n-tile partition size for layer-1 output
NNT = N1 // PN              # n tiles
SH = 512                    # PSUM-bank half of the sample axis
ACT_KS = frozenset((0, 2, 4, 6, 8, 10, 11))  # passes on ScalarE; rest on VectorE

f32 = mybir.dt.float32
f32r = mybir.dt.float32r

LAST_RUN = {}
_CACHE = {}


def _build_program():
    nc = bacc.Bacc("TRN2", target_bir_lowering=False, debug=False,
                   num_devices=NCORES)
    d = {}

    def inp(name, shape, dt=f32):
        d[name] = nc.dram_tensor(name, list(shape), dt, kind="ExternalInput").ap()

    inp("xT", (NGT, PT, T * S), mybir.dt.bfloat16)
    inp("scl", (PT, NGT * NK))
    inp("bia", (PT, NGT * NK))
    inp("cst", (PT, NGT))
    inp("ident", (PT, PT))
    inp("coe", (PT, NGT * NK))
    inp("cw0t", (NNT, PT, NGT * PN), f32r)
    inp("cb0", (PN, 2))
    inp("cw1t", (PN, 2 * N2), f32r)
    inp("cb1", (100, 2))
    inp("cw2t", (N2, N3), f32r)
    inp("cb2", (N3, 1))
    inp("cwft", (N3, 1), f32r)
    inp("cbf", (1, 1))
    out_d = nc.dram_tensor("out", [1, S], f32, kind="ExternalOutput").ap()

    Relu = mybir.ActivationFunctionType.Relu
    Ident = mybir.ActivationFunctionType.Identity

    with tile.TileContext(nc) as tc, ExitStack() as ctx:
        const = ctx.enter_context(tc.tile_pool(name="const", bufs=1))
        xpool = ctx.enter_context(tc.tile_pool(name="x", bufs=12))
        dpool = ctx.enter_context(tc.tile_pool(name="diag", bufs=3))
        apool = ctx.enter_context(tc.tile_pool(name="a", bufs=3))
        vpool = ctx.enter_context(tc.tile_pool(name="v", bufs=2))
        zpool = ctx.enter_context(tc.tile_pool(name="z", bufs=NGT))
        wpool = ctx.enter_context(tc.tile_pool(name="w0", bufs=4))
        opool = ctx.enter_context(tc.tile_pool(name="o1", bufs=2))
        tpool = ctx.enter_context(tc.tile_pool(name="tail", bufs=1))
        zps = ctx.enter_context(tc.tile_pool(name="zps", bufs=4, space="PSUM"))
        mmps = ctx.enter_context(tc.tile_pool(name="mmps", bufs=4, space="PSUM"))
        dram = ctx.enter_context(tc.tile_pool(name="dram", bufs=1, space="DRAM"))

        # x preload for the first two gene tiles ahead of everything else
        # (HWDGE drains FIFO per engine; these gate the phase-A ramp).
        x_pre = {}
        for gt in range(2):
            for t in range(T):
                xt = xpool.tile([PT, S], mybir.dt.bfloat16, tag="x",
                                name=f"x{gt}_{t}")
                (nc.gpsimd, nc.sync, nc.gpsimd)[(gt * T + t) % 3].dma_start(
                    xt[:], d["xT"][gt, :, t * S:(t + 1) * S])
                x_pre[(gt, t)] = xt

        sclt = const.tile([PT, NGT * NK], f32)
        nc.scalar.dma_start(sclt[:], d["scl"][:])
        identt = const.tile([PT, PT], f32)
        nc.scalar.dma_start(identt[:], d["ident"][:])
        coet = const.tile([PT, NGT * NK], f32)
        nc.scalar.dma_start(coet[:], d["coe"][:])
        biat = const.tile([PT, NGT * NK], f32)
        nc.scalar.dma_start(biat[:], d["bia"][:])
        cstt = const.tile([PT, NGT], f32)
        nc.scalar.dma_start(cstt[:], d["cst"][:])
        cb0t = const.tile([PN, 2], f32)
        nc.scalar.dma_start(cb0t[:], d["cb0"][:])
        w1t = const.tile([PN, 2 * N2], f32r)
        nc.scalar.dma_start(w1t[:], d["cw1t"][:])
        cb1t = const.tile([100, 2], f32)
        nc.scalar.dma_start(cb1t[:], d["cb1"][:])
        cb2t = const.tile([N3, 1], f32)
        nc.scalar.dma_start(cb2t[:], d["cb2"][:])
        cwftt = const.tile([N3, 1], f32r)
        nc.scalar.dma_start(cwftt[:], d["cwft"][:])
        cbft = const.tile([1, 1], f32)
        nc.scalar.dma_start(cbft[:], d["cbf"][:])
        cw2tt = const.tile([100, 2 * N3], f32r)
        for mc in range(2):
            nc.scalar.dma_start(cw2tt[:, mc * N3:(mc + 1) * N3],
                              d["cw2t"][mc * 100:(mc + 1) * 100, :])

        NCH = 2                       # reduce-scatter chunks (1000 rows each)
        partial_c = [dram.tile([N1 // NCH, S], f32, tag=f"pc{j}",
                               name=f"partial{j}") for j in range(NCH)]
        rs_c = [dram.tile([PN, S], f32, tag=f"rs{j}",
                          name=f"rsout{j}") for j in range(NCH)]
        partial2 = [dram.tile([N2, S], f32, tag=f"p2{j}",
                               name=f"partial2{j}") for j in range(NCH)]
        summed2 = [dram.tile([N2, S], f32, tag=f"s2{j}",
                             name=f"summed2{j}") for j in range(NCH)]
        ccwarm_in = dram.tile([1, 128], f32, tag="ccwi")
        ccwarm_out = dram.tile([1, 128], f32, tag="ccwo")

        # tiny warm-up collective: absorbs the first-rendezvous / ncfw
        # cold-start cost during phase A instead of on the critical tail
        ccwarm_sb = const.tile([1, 128], f32)
        nc.gpsimd.memset(ccwarm_sb[:], 0.0)
        nc.gpsimd.dma_start(ccwarm_in[:], ccwarm_sb[:])
        nc.gpsimd.collective_compute(
            "AllReduce", mybir.AluOpType.add,
            replica_groups=[list(range(NCORES))],
            ins=[ccwarm_in.opt()], outs=[ccwarm_out.opt()],
        )


        # w0 prefetch: first three layer-1 weight blocks load during phase A
        # (HWDGE queues drain FIFO per engine, so these must precede the
        # phase-A loads in program order to be ready when phase B starts).
        HW0 = NGT * PN // 2

        def load_w0(nt):
            w = wpool.tile([PT, NGT * PN], f32r, tag="w0", name=f"w0_{nt}")
            nc.scalar.dma_start(w[:, :HW0], d["cw0t"][nt, :, :HW0])
            nc.gpsimd.dma_start(w[:, HW0:], d["cw0t"][nt, :, HW0:])
            return w

        w_tiles = {nt: load_w0(nt) for nt in range(3)}

        # ---------------- Phase A: local gene MLPs + combinor ----------------
        z_tiles = []
        for gt in range(NGT):
            if gt < 2:
                xts = [x_pre[(gt, t)] for t in range(T)]
            else:
                xts = []
                for t in range(T):
                    xt = xpool.tile([PT, S], mybir.dt.bfloat16, tag="x",
                                    name=f"x{gt}_{t}")
                    (nc.gpsimd, nc.sync, nc.gpsimd)[(gt * T + t) % 3].dma_start(
                        xt[:], d["xT"][gt, :, t * S:(t + 1) * S])
                    xts.append(xt)
            pss = (zps.tile([PT, SH], f32, tag="zps", name=f"zps{gt}_0"),
                   zps.tile([PT, SH], f32, tag="zps", name=f"zps{gt}_1"))
            for k in range(NK):
                t = k // H
                ci = gt * NK + k
                if k in ACT_KS:
                    a = apool.tile([PT, S], f32r, tag="a")
                    nc.scalar.activation(a[:], xts[t], Relu,
                                         bias=biat[:, ci:ci + 1],
                                         scale=sclt[:, ci:ci + 1])
                else:
                    v = vpool.tile([PT, S], f32, tag="v")
                    nc.vector.tensor_scalar(v[:], xts[t],
                                            sclt[:, ci:ci + 1],
                                            biat[:, ci:ci + 1],
                                            mybir.AluOpType.mult,
                                            mybir.AluOpType.add)
                    a = apool.tile([PT, S], f32r, tag="a")
                    nc.vector.tensor_scalar(a[:], v[:], 0.0, None,
                                            mybir.AluOpType.max)
                dg = dpool.tile([PT, PT], f32r, tag="diag", name=f"dg{gt}_{k}")
                nc.vector.tensor_scalar(dg[:], identt[:], coet[:, ci:ci + 1],
                                        None, mybir.AluOpType.mult)
                for sh in range(2):
                    nc.tensor.matmul(pss[sh][:], dg[:],
                                     a[:, sh * SH:(sh + 1) * SH],
                                     start=(k == 0), stop=(k == NK - 1))
            z = zpool.tile([PT, S], f32r, tag="z")
            for sh in range(2):
                nc.scalar.activation(z[:, sh * SH:(sh + 1) * SH], pss[sh][:],
                                     Ident, bias=cstt[:, gt:gt + 1], scale=1.0)
            z_tiles.append(z)

        # ---------------- Phase B: out1T = CW0 @ z (local-gene partial) ------
        # chunked: as each 500-row quarter of the partial is stored, its
        # AllReduce is issued so comm overlaps the remaining matmuls.
        partial_cr = [p[:].rearrange("(g p) s -> p g s", p=PN)
                      for p in partial_c]
        for ng in range(NNT // 2):
            o = opool.tile([PN, 2 * S], f32, tag="o1")
            for j in range(2):
                nt = ng * 2 + j
                if nt in w_tiles:
                    w = w_tiles.pop(nt)
                else:
                    w = load_w0(nt)
                pp = (mmps.tile([PN, SH], f32, tag="mm", name=f"mm{nt}_0"),
                      mmps.tile([PN, SH], f32, tag="mm", name=f"mm{nt}_1"))
                for gt in range(NGT):
                    for sh in range(2):
                        nc.tensor.matmul(pp[sh][:],
                                         w[:, gt * PN:(gt + 1) * PN],
                                         z_tiles[gt][:, sh * SH:(sh + 1) * SH],
                                         start=(gt == 0), stop=(gt == NGT - 1))
                for sh in range(2):
                    nc.scalar.copy(o[:, j * S + sh * SH:j * S + (sh + 1) * SH],
                                   pp[sh][:])
            j, ng2 = divmod(ng, 4)
            nc.sync.dma_start(
                partial_cr[j][:, ng2 * 2:(ng2 + 1) * 2, :], o[:])
            if ng2 == 3:
                nc.gpsimd.collective_compute(
                    "ReduceScatter", mybir.AluOpType.add,
                    replica_groups=[list(range(NCORES))],
                    ins=[partial_c[j].opt()], outs=[rs_c[j].opt()],
                )

        # ------- Phase D: distributed 2000->200 (each core owns 250 rows of
        # the 2000-dim layer via ReduceScatter), then AllReduce the small
        # (200, S) partial and finish 200->20->1 replicated. -------------
        z1_tiles = []
        for j in range(NCH):
            y1 = apool.tile([PN, S], f32, tag="a", name=f"y1_{j}")
            nc.sync.dma_start(y1[:], rs_c[j][:])
            z1 = zpool.tile([PN, S], f32r, tag="z", name=f"z1_{j}")
            nc.scalar.activation(z1[:], y1[:], Relu,
                                 bias=cb0t[:, j:j + 1], scale=1.0)
            z1_tiles.append(z1)
            for mc in range(2):
                o2 = opool.tile([100, S], f32, tag="o1", name=f"o2_{j}{mc}")
                for sh in range(2):
                    ps = mmps.tile([100, SH], f32, tag="mm",
                                   name=f"ps2_{j}{mc}{sh}")
                    nc.tensor.matmul(
                        ps[:], w1t[:, j * N2 + mc * 100:j * N2 + (mc + 1) * 100],
                        z1[:, sh * SH:(sh + 1) * SH], start=True, stop=True)
                    nc.scalar.copy(o2[:, sh * SH:(sh + 1) * SH], ps[:])
                nc.sync.dma_start(partial2[j][mc * 100:(mc + 1) * 100, :], o2[:])
            nc.gpsimd.collective_compute(
                "AllReduce", mybir.AluOpType.add,
                replica_groups=[list(range(NCORES))],
                ins=[partial2[j].opt()], outs=[summed2[j].opt()],
            )
        z2all = tpool.tile([100, 2 * S], f32r, tag="z2")
        z2_tiles = [z2all[:, 0:S], z2all[:, S:2 * S]]
        for mc in range(2):
            y2a = apool.tile([100, S], f32, tag="a", name=f"y2a_{mc}")
            nc.sync.dma_start(y2a[:], summed2[0][mc * 100:(mc + 1) * 100, :])
            y2b = apool.tile([100, S], f32, tag="a", name=f"y2b_{mc}")
            nc.scalar.dma_start(y2b[:], summed2[1][mc * 100:(mc + 1) * 100, :])
            nc.vector.tensor_tensor(y2a[:], y2a[:], y2b[:],
                                    mybir.AluOpType.add)
            nc.scalar.activation(z2_tiles[mc][:], y2a[:], Relu,
                                 bias=cb1t[:, mc:mc + 1], scale=1.0)
        z3 = tpool.tile([N3, S], f32r, tag="z3")
        for sh in range(2):
            ps = mmps.tile([N3, SH], f32, tag="mm")
            for mc in range(2):
                nc.tensor.matmul(ps[:], cw2tt[:, mc * N3:(mc + 1) * N3],
                                 z2_tiles[mc][:, sh * SH:(sh + 1) * SH],
                                 start=(mc == 0), stop=(mc == 1))
            nc.scalar.activation(z3[:, sh * SH:(sh + 1) * SH], ps[:], Relu,
                                 bias=cb2t[:], scale=1.0)
        outt = tpool.tile([1, S], f32, tag="outt")
        for sh in range(2):
            ps = mmps.tile([1, SH], f32, tag="mm")
            nc.tensor.matmul(ps[:], cwftt[:],
                             z3[:, sh * SH:(sh + 1) * SH],
                             start=True, stop=True)
            nc.scalar.activation(outt[:, sh * SH:(sh + 1) * SH], ps[:], Ident,
                                 bias=cbft[:], scale=1.0)
        nc.sync.dma_start(out_d[:], outt[:])

    nc.compile()
    return nc


def _shard_inputs(x, W1, b1, W2, b2, Wc, bc,
                  CW0, Cb0, CW1, Cb1, CW2, Cb2, CWf, Cbf):
    f = lambda a: np.ascontiguousarray(a, dtype=np.float32)
    CW1T = np.ascontiguousarray(CW1.T)
    shared = {
        "cb1": f(Cb1.reshape(2, 100).T),
        "cw2t": f(CW2.T),
        "cb2": f(Cb2.reshape(N3, 1)),
        "cwft": f(CWf.T),
        "cbf": f(Cbf.reshape(1, 1)),
    }
    in_maps = []
    for c in range(NCORES):
        gs = slice(c * GL, (c + 1) * GL)
        scl = W1[:, gs, :].transpose(1, 0, 2).reshape(GL, NK)
        bia = b1[:, gs, :].transpose(1, 0, 2).reshape(GL, NK)
        coe = (W2[:, gs, :] * Wc[gs, :].T[:, :, None]) \
            .transpose(1, 0, 2).reshape(GL, NK)
        cst = (b2[:, gs] * Wc[gs, :].T).sum(0) + bc[gs]
        half = N1 // 2
        in_maps.append({
            "cb0": f(np.stack([Cb0[c * PN:(c + 1) * PN],
                               Cb0[half + c * PN:half + (c + 1) * PN]], axis=1)),
            "cw1t": f(np.concatenate(
                [CW1T[c * PN:(c + 1) * PN, :],
                 CW1T[half + c * PN:half + (c + 1) * PN, :]], axis=1)),
            "xT": np.ascontiguousarray(
                x[:, :, gs].transpose(2, 0, 1).reshape(NGT, PT, T * S)
            ).astype(ml_dtypes.bfloat16),
            "scl": f(scl.reshape(NGT, PT, NK).transpose(1, 0, 2)
                     .reshape(PT, NGT * NK)),
            "bia": f(bia.reshape(NGT, PT, NK).transpose(1, 0, 2)
                     .reshape(PT, NGT * NK)),
            "cst": f(cst.reshape(NGT, PT).T),
            "ident": np.eye(PT, dtype=np.float32),
            "coe": f(coe.reshape(NGT, PT, NK).transpose(1, 0, 2)
                     .reshape(PT, NGT * NK)),
            "cw0t": f(CW0[:, gs].reshape(NNT, PN, NGT, PT)
                      .transpose(0, 3, 2, 1).reshape(NNT, PT, NGT * PN)),
            **shared,
        })
    return in_maps


def _install_profile_shim():
    """Register the NTFF profiling hook that this container's antenv lacks.

    bass_utils' trace path imports antenv.axon_hooks; the boot helper that
    can construct the actual hook exists, so wire it up dynamically.
    """
    import types
    try:
        import antenv.axon_hooks  # noqa: F401
        return True
    except ImportError:
        pass
    try:
        import antenv
        from trn_agent_boot.trn_boot import _ntff_profile_via_ctypes
        hook = _ntff_profile_via_ctypes("/opt/axon/libaxon_pjrt.so")
        mod = types.ModuleType("antenv.axon_hooks")
        mod.get_axon_ntff_profile_hook = lambda: hook
        mod.set_axon_ntff_profile_hook = lambda h: None
        sys.modules["antenv.axon_hooks"] = mod
        antenv.axon_hooks = mod
        return hook is not None
    except Exception:
        return False


def kernel(**inputs):
    inputs = {k: np.asarray(v) for k, v in inputs.items()}
    in_maps = _shard_inputs(**inputs)
    if "nc" not in _CACHE:
        _CACHE["nc"] = _build_program()
    nc = _CACHE["nc"]
    trace = bool(os.environ.get("KERNEL_PROFILE")) and _install_profile_shim()
    res = run_bass_kernel_spmd(nc, in_maps, core_ids=list(range(NCORES)),
                               trace=trace)
    LAST_RUN["exec_time_ns"] = res.exec_time_ns
    LAST_RUN["mean_exec_time_ns"] = res.mean_exec_time_ns
    if res.instructions_and_trace is not None:
        LAST_RUN["trace_path"] = res.instructions_and_trace[1]
    return res.results[0]["out"].reshape(1, S, 1)


if __name__ == "__main__":
    rng = np.random.default_rng(0)
    ins = {
        "x": rng.standard_normal((T, S, G), dtype=np.float32),
        "W1": rng.standard_normal((T, G, H), dtype=np.float32) * 0.5,
        "b1": rng.standard_normal((T, G, H), dtype=np.float32) * 0.1,
        "W2": rng.standard_normal((T, G, H), dtype=np.float32) * 0.5,
        "b2": rng.standard_normal((T, G), dtype=np.float32) * 0.1,
        "Wc": rng.standard_normal((G, T), dtype=np.float32) * 0.5,
        "bc": rng.standard_normal((G,), dtype=np.float32) * 0.1,
        "CW0": rng.standard_normal((N1, G), dtype=np.float32) * 0.007,
        "Cb0": rng.standard_normal((N1,), dtype=np.float32) * 0.007,
        "CW1": rng.standard_normal((N2, N1), dtype=np.float32) * 0.02,
        "Cb1": rng.standard_normal((N2,), dtype=np.float32) * 0.02,
        "CW2": rng.standard_normal((N3, N2), dtype=np.float32) * 0.07,
        "Cb2": rng.standard_normal((N3,), dtype=np.float32) * 0.07,
        "CWf": rng.standard_normal((1, N3), dtype=np.float32) * 0.2,
        "Cbf": rng.standard_normal((1,), dtype=np.float32) * 0.2,
    }
    out = kernel(**ins)
    # numpy reference
    xx = ins["x"]
    h = np.maximum(xx[..., None] * ins["W1"][:, None] + ins["b1"][:, None], 0.0)
    y = np.einsum("tsgh,tgh->tsg", h, ins["W2"]) + ins["b2"][:, None, :]
    zz = np.einsum("tsg,gt->sg", y, ins["Wc"]) + ins["bc"]
    for Wl, bl in ((ins["CW0"], ins["Cb0"]), (ins["CW1"], ins["Cb1"]),
                   (ins["CW2"], ins["Cb2"])):
        zz = np.maximum(zz @ Wl.T + bl, 0.0)
    ref = (zz @ ins["CWf"].T + ins["Cbf"])[None]
    err = np.abs(out - ref).max() / (np.abs(ref).max() + 1e-12)
    print("self-test rel err:", err)
    print("exec_time_ns:", LAST_RUN.get("exec_time_ns"))


# revision 24
# speedup vs baseline: 1.0216x; 1.0216x over previous
"""Trainium2 Bass kernel for nn_CombinedAMLModel (dense_mlp, 8 NeuronCores).

Sharding: tensor-parallel over the gene axis (20000 genes -> 2500 per core).

Per core:
  Phase A  - per-(tech,gene) 1->4->1 MLPs plus the per-gene tech combinor,
             computed as 12 relu-affine passes (genes on partitions, per-
             partition scale/bias on ACT/DVE), accumulated into PSUM with
             diagonal fp32r matmuls whose diagonals carry W2[t,g,h]*Wc[g,t].
             The constant term (sum_t b2*Wc + bc) is added during the
             PSUM->SBUF copy. Produces z[g_local, s] (2500 x 1024).
  Phase B  - out1T[n, s] += CW0T[g, n].T @ z[g, s]  (fp32r, K=2500 local
             genes, n=2000), written to DRAM as this core's partial.
  Phase C  - AllReduce of the (2000, 1024) partials across 8 cores.
  Phase D  - tail MLP 2000->200->20->1, replicated on every core, computed
             entirely in transposed orientation (layer outputs on partitions,
             samples on the free axis) so no transposes are needed anywhere.

All matmuls run in float32r (full-rate fp32 matmul, ~1e-4 relative error).
"""
import os
import sys

sys.path.insert(0, "/opt/trn_rl_repo")

import ml_dtypes
import numpy as np
from contextlib import ExitStack

import concourse.bass as bass
import concourse.tile as tile
from concourse import bacc, mybir
from concourse.bass_utils import run_bass_kernel_spmd

T, S, G, H = 3, 1024, 20000, 4
NCORES = 8
GL = G // NCORES            # genes per core
PT = 125                    # gene-tile partition size
NGT = GL // PT              # gene tiles per core
NK = T * H                  # local relu-affine passes
N1, N2, N3 = 2000, 200, 20
PN = 125                    # n-tile partition size for layer-1 output
NNT = N1 // PN              # n tiles
SH = 512                    # PSUM-bank half of the sample axis
ACT_KS = frozenset((0, 2, 4, 6, 8, 10, 11))  # passes on ScalarE; rest on VectorE

f32 = mybir.dt.float32
f32r = mybir.dt.float32r

LAST_RUN = {}
_CACHE = {}


def _build_program():
    nc = bacc.Bacc("TRN2", target_bir_lowering=False, debug=False,
                   num_devices=NCORES)
    d = {}

    def inp(name, shape, dt=f32):
        d[name] = nc.dram_tensor(name, list(shape), dt, kind="ExternalInput").ap()

    inp("xT", (NGT, PT, T * S), mybir.dt.bfloat16)
    inp("scl", (PT, NGT * NK))
    inp("bia", (PT, NGT * NK))
    inp("cst", (PT, NGT))
    inp("ident", (PT, PT))
    inp("coe", (PT, NGT * NK))
    inp("cw0t", (NNT, PT, NGT * PN), mybir.dt.bfloat16)
    inp("cb0", (PN, 2))
    inp("cw1t", (PN, 2 * N2), f32r)
    inp("cb1", (100, 2))
    inp("cw2t", (N2, N3), f32r)
    inp("cb2", (N3, 1))
    inp("cwft", (N3, 1), f32r)
    inp("cbf", (1, 1))
    out_d = nc.dram_tensor("out", [1, S], f32, kind="ExternalOutput").ap()

    Relu = mybir.ActivationFunctionType.Relu
    Ident = mybir.ActivationFunctionType.Identity

    with tile.TileContext(nc) as tc, ExitStack() as ctx:
        const = ctx.enter_context(tc.tile_pool(name="const", bufs=1))
        xpool = ctx.enter_context(tc.tile_pool(name="x", bufs=12))
        dpool = ctx.enter_context(tc.tile_pool(name="diag", bufs=3))
        apool = ctx.enter_context(tc.tile_pool(name="a", bufs=3))
        vpool = ctx.enter_context(tc.tile_pool(name="v", bufs=2))
        zpool = ctx.enter_context(tc.tile_pool(name="z", bufs=NGT))
        wpool = ctx.enter_context(tc.tile_pool(name="w0", bufs=4))
        opool = ctx.enter_context(tc.tile_pool(name="o1", bufs=2))
        tpool = ctx.enter_context(tc.tile_pool(name="tail", bufs=1))
        zps = ctx.enter_context(tc.tile_pool(name="zps", bufs=4, space="PSUM"))
        mmps = ctx.enter_context(tc.tile_pool(name="mmps", bufs=4, space="PSUM"))
        dram = ctx.enter_context(tc.tile_pool(name="dram", bufs=1, space="DRAM"))

        # x preload for the first two gene tiles ahead of everything else
        # (HWDGE drains FIFO per engine; these gate the phase-A ramp).
        x_pre = {}
        for gt in range(2):
            for t in range(T):
                xt = xpool.tile([PT, S], mybir.dt.bfloat16, tag="x",
                                name=f"x{gt}_{t}")
                (nc.gpsimd, nc.sync, nc.gpsimd)[(gt * T + t) % 3].dma_start(
                    xt[:], d["xT"][gt, :, t * S:(t + 1) * S])
                x_pre[(gt, t)] = xt

        sclt = const.tile([PT, NGT * NK], f32)
        nc.scalar.dma_start(sclt[:], d["scl"][:])
        identt = const.tile([PT, PT], f32)
        nc.scalar.dma_start(identt[:], d["ident"][:])
        coet = const.tile([PT, NGT * NK], f32)
        nc.scalar.dma_start(coet[:], d["coe"][:])
        biat = const.tile([PT, NGT * NK], f32)
        nc.scalar.dma_start(biat[:], d["bia"][:])
        cstt = const.tile([PT, NGT], f32)
        nc.scalar.dma_start(cstt[:], d["cst"][:])
        cb0t = const.tile([PN, 2], f32)
        nc.scalar.dma_start(cb0t[:], d["cb0"][:])
        w1t = const.tile([PN, 2 * N2], f32r)
        nc.scalar.dma_start(w1t[:], d["cw1t"][:])
        cb1t = const.tile([100, 2], f32)
        nc.scalar.dma_start(cb1t[:], d["cb1"][:])
        cb2t = const.tile([N3, 1], f32)
        nc.scalar.dma_start(cb2t[:], d["cb2"][:])
        cwftt = const.tile([N3, 1], f32r)
        nc.scalar.dma_start(cwftt[:], d["cwft"][:])
        cbft = const.tile([1, 1], f32)
        nc.scalar.dma_start(cbft[:], d["cbf"][:])
        cw2tt = const.tile([100, 2 * N3], f32r)
        for mc in range(2):
            nc.scalar.dma_start(cw2tt[:, mc * N3:(mc + 1) * N3],
                              d["cw2t"][mc * 100:(mc + 1) * 100, :])

        NCH = 2                       # reduce-scatter chunks (1000 rows each)
        partial_c = [dram.tile([N1 // NCH, S], f32, tag=f"pc{j}",
                               name=f"partial{j}") for j in range(NCH)]
        rs_c = [dram.tile([PN, S], f32, tag=f"rs{j}",
                          name=f"rsout{j}") for j in range(NCH)]
        partial2 = [dram.tile([N2, S], f32, tag=f"p2{j}",
                               name=f"partial2{j}") for j in range(NCH)]
        summed2 = [dram.tile([N2, S], f32, tag=f"s2{j}",
                             name=f"summed2{j}") for j in range(NCH)]
        ccwarm_in = dram.tile([1, 128], f32, tag="ccwi")
        ccwarm_out = dram.tile([1, 128], f32, tag="ccwo")

        # tiny warm-up collective: absorbs the first-rendezvous / ncfw
        # cold-start cost during phase A instead of on the critical tail
        ccwarm_sb = const.tile([1, 128], f32)
        nc.gpsimd.memset(ccwarm_sb[:], 0.0)
        nc.gpsimd.dma_start(ccwarm_in[:], ccwarm_sb[:])
        nc.gpsimd.collective_compute(
            "AllReduce", mybir.AluOpType.add,
            replica_groups=[list(range(NCORES))],
            ins=[ccwarm_in.opt()], outs=[ccwarm_out.opt()],
        )


        # w0 prefetch: first three layer-1 weight blocks load during phase A
        # (HWDGE queues drain FIFO per engine, so these must precede the
        # phase-A loads in program order to be ready when phase B starts).
        HW0 = NGT * PN // 2

        def load_w0(nt):
            w = wpool.tile([PT, NGT * PN], mybir.dt.bfloat16, tag="w0", name=f"w0_{nt}")
            nc.scalar.dma_start(w[:, :HW0], d["cw0t"][nt, :, :HW0])
            nc.gpsimd.dma_start(w[:, HW0:], d["cw0t"][nt, :, HW0:])
            return w

        w_tiles = {nt: load_w0(nt) for nt in range(3)}

        # ---------------- Phase A: local gene MLPs + combinor ----------------
        z_tiles = []
        for gt in range(NGT):
            if gt < 2:
                xts = [x_pre[(gt, t)] for t in range(T)]
            else:
                xts = []
                for t in range(T):
                    xt = xpool.tile([PT, S], mybir.dt.bfloat16, tag="x",
                                    name=f"x{gt}_{t}")
                    (nc.gpsimd, nc.sync, nc.gpsimd)[(gt * T + t) % 3].dma_start(
                        xt[:], d["xT"][gt, :, t * S:(t + 1) * S])
                    xts.append(xt)
            pss = (zps.tile([PT, SH], f32, tag="zps", name=f"zps{gt}_0"),
                   zps.tile([PT, SH], f32, tag="zps", name=f"zps{gt}_1"))
            for k in range(NK):
                t = k // H
                ci = gt * NK + k
                if k in ACT_KS:
                    a = apool.tile([PT, S], f32r, tag="a")
                    nc.scalar.activation(a[:], xts[t], Relu,
                                         bias=biat[:, ci:ci + 1],
                                         scale=sclt[:, ci:ci + 1])
                else:
                    v = vpool.tile([PT, S], f32, tag="v")
                    nc.vector.tensor_scalar(v[:], xts[t],
                                            sclt[:, ci:ci + 1],
                                            biat[:, ci:ci + 1],
                                            mybir.AluOpType.mult,
                                            mybir.AluOpType.add)
                    a = apool.tile([PT, S], f32r, tag="a")
                    nc.vector.tensor_scalar(a[:], v[:], 0.0, None,
                                            mybir.AluOpType.max)
                dg = dpool.tile([PT, PT], f32r, tag="diag", name=f"dg{gt}_{k}")
                nc.vector.tensor_scalar(dg[:], identt[:], coet[:, ci:ci + 1],
                                        None, mybir.AluOpType.mult)
                for sh in range(2):
                    nc.tensor.matmul(pss[sh][:], dg[:],
                                     a[:, sh * SH:(sh + 1) * SH],
                                     start=(k == 0), stop=(k == NK - 1))
            z = zpool.tile([PT, S], mybir.dt.bfloat16, tag="z")
            for sh in range(2):
                nc.scalar.activation(z[:, sh * SH:(sh + 1) * SH], pss[sh][:],
                                     Ident, bias=cstt[:, gt:gt + 1], scale=1.0)
            z_tiles.append(z)

        # ---------------- Phase B: out1T = CW0 @ z (local-gene partial) ------
        # chunked: as each 500-row quarter of the partial is stored, its
        # AllReduce is issued so comm overlaps the remaining matmuls.
        partial_cr = [p[:].rearrange("(g p) s -> p g s", p=PN)
                      for p in partial_c]
        for ng in range(NNT // 2):
            o = opool.tile([PN, 2 * S], f32, tag="o1")
            for j in range(2):
                nt = ng * 2 + j
                if nt in w_tiles:
                    w = w_tiles.pop(nt)
                else:
                    w = load_w0(nt)
                pp = (mmps.tile([PN, SH], f32, tag="mm", name=f"mm{nt}_0"),
                      mmps.tile([PN, SH], f32, tag="mm", name=f"mm{nt}_1"))
                for gt in range(NGT):
                    for sh in range(2):
                        nc.tensor.matmul(pp[sh][:],
                                         w[:, gt * PN:(gt + 1) * PN],
                                         z_tiles[gt][:, sh * SH:(sh + 1) * SH],
                                         start=(gt == 0), stop=(gt == NGT - 1))
                for sh in range(2):
                    nc.scalar.copy(o[:, j * S + sh * SH:j * S + (sh + 1) * SH],
                                   pp[sh][:])
            j, ng2 = divmod(ng, 4)
            nc.sync.dma_start(
                partial_cr[j][:, ng2 * 2:(ng2 + 1) * 2, :], o[:])
            if ng2 == 3:
                nc.gpsimd.collective_compute(
                    "ReduceScatter", mybir.AluOpType.add,
                    replica_groups=[list(range(NCORES))],
                    ins=[partial_c[j].opt()], outs=[rs_c[j].opt()],
                )

        # ------- Phase D: distributed 2000->200 (each core owns 250 rows of
        # the 2000-dim layer via ReduceScatter), then AllReduce the small
        # (200, S) partial and finish 200->20->1 replicated. -------------
        z1_tiles = []
        for j in range(NCH):
            y1 = apool.tile([PN, S], f32, tag="a", name=f"y1_{j}")
            nc.sync.dma_start(y1[:], rs_c[j][:])
            z1 = tpool.tile([PN, S], f32r, tag=f"z1_{j}", name=f"z1_{j}")
            nc.scalar.activation(z1[:], y1[:], Relu,
                                 bias=cb0t[:, j:j + 1], scale=1.0)
            z1_tiles.append(z1)
            for mc in range(2):
                o2 = opool.tile([100, S], f32, tag="o1", name=f"o2_{j}{mc}")
                for sh in range(2):
                    ps = mmps.tile([100, SH], f32, tag="mm",
                                   name=f"ps2_{j}{mc}{sh}")
                    nc.tensor.matmul(
                        ps[:], w1t[:, j * N2 + mc * 100:j * N2 + (mc + 1) * 100],
                        z1[:, sh * SH:(sh + 1) * SH], start=True, stop=True)
                    nc.scalar.copy(o2[:, sh * SH:(sh + 1) * SH], ps[:])
                nc.sync.dma_start(partial2[j][mc * 100:(mc + 1) * 100, :], o2[:])
            nc.gpsimd.collective_compute(
                "AllReduce", mybir.AluOpType.add,
                replica_groups=[list(range(NCORES))],
                ins=[partial2[j].opt()], outs=[summed2[j].opt()],
            )
        z2all = tpool.tile([100, 2 * S], f32r, tag="z2")
        z2_tiles = [z2all[:, 0:S], z2all[:, S:2 * S]]
        for mc in range(2):
            y2a = apool.tile([100, S], f32, tag="a", name=f"y2a_{mc}")
            nc.sync.dma_start(y2a[:], summed2[0][mc * 100:(mc + 1) * 100, :])
            y2b = apool.tile([100, S], f32, tag="a", name=f"y2b_{mc}")
            nc.scalar.dma_start(y2b[:], summed2[1][mc * 100:(mc + 1) * 100, :])
            nc.vector.tensor_tensor(y2a[:], y2a[:], y2b[:],
                                    mybir.AluOpType.add)
            nc.scalar.activation(z2_tiles[mc][:], y2a[:], Relu,
                                 bias=cb1t[:, mc:mc + 1], scale=1.0)
        z3 = tpool.tile([N3, S], f32r, tag="z3")
        for sh in range(2):
            ps = mmps.tile([N3, SH], f32, tag="mm")
            for mc in range(2):
                nc.tensor.matmul(ps[:], cw2tt[:, mc * N3:(mc + 1) * N3],
                                 z2_tiles[mc][:, sh * SH:(sh + 1) * SH],
                                 start=(mc == 0), stop=(mc == 1))
            nc.scalar.activation(z3[:, sh * SH:(sh + 1) * SH], ps[:], Relu,
                                 bias=cb2t[:], scale=1.0)
        outt = tpool.tile([1, S], f32, tag="outt")
        for sh in range(2):
            ps = mmps.tile([1, SH], f32, tag="mm")
            nc.tensor.matmul(ps[:], cwftt[:],
                             z3[:, sh * SH:(sh + 1) * SH],
                             start=True, stop=True)
            nc.scalar.activation(outt[:, sh * SH:(sh + 1) * SH], ps[:], Ident,
                                 bias=cbft[:], scale=1.0)
        nc.sync.dma_start(out_d[:], outt[:])

    nc.compile()
    return nc


def _shard_inputs(x, W1, b1, W2, b2, Wc, bc,
                  CW0, Cb0, CW1, Cb1, CW2, Cb2, CWf, Cbf):
    f = lambda a: np.ascontiguousarray(a, dtype=np.float32)
    CW1T = np.ascontiguousarray(CW1.T)
    shared = {
        "cb1": f(Cb1.reshape(2, 100).T),
        "cw2t": f(CW2.T),
        "cb2": f(Cb2.reshape(N3, 1)),
        "cwft": f(CWf.T),
        "cbf": f(Cbf.reshape(1, 1)),
    }
    in_maps = []
    for c in range(NCORES):
        gs = slice(c * GL, (c + 1) * GL)
        scl = W1[:, gs, :].transpose(1, 0, 2).reshape(GL, NK)
        bia = b1[:, gs, :].transpose(1, 0, 2).reshape(GL, NK)
        coe = (W2[:, gs, :] * Wc[gs, :].T[:, :, None]) \
            .transpose(1, 0, 2).reshape(GL, NK)
        cst = (b2[:, gs] * Wc[gs, :].T).sum(0) + bc[gs]
        half = N1 // 2
        in_maps.append({
            "cb0": f(np.stack([Cb0[c * PN:(c + 1) * PN],
                               Cb0[half + c * PN:half + (c + 1) * PN]], axis=1)),
            "cw1t": f(np.concatenate(
                [CW1T[c * PN:(c + 1) * PN, :],
                 CW1T[half + c * PN:half + (c + 1) * PN, :]], axis=1)),
            "xT": np.ascontiguousarray(
                x[:, :, gs].transpose(2, 0, 1).reshape(NGT, PT, T * S)
            ).astype(ml_dtypes.bfloat16),
            "scl": f(scl.reshape(NGT, PT, NK).transpose(1, 0, 2)
                     .reshape(PT, NGT * NK)),
            "bia": f(bia.reshape(NGT, PT, NK).transpose(1, 0, 2)
                     .reshape(PT, NGT * NK)),
            "cst": f(cst.reshape(NGT, PT).T),
            "ident": np.eye(PT, dtype=np.float32),
            "coe": f(coe.reshape(NGT, PT, NK).transpose(1, 0, 2)
                     .reshape(PT, NGT * NK)),
            "cw0t": np.ascontiguousarray(
                CW0[:, gs].reshape(NNT, PN, NGT, PT)
                .transpose(0, 3, 2, 1).reshape(NNT, PT, NGT * PN)
            ).astype(ml_dtypes.bfloat16),
            **shared,
        })
    return in_maps


def _install_profile_shim():
    """Register the NTFF profiling hook that this container's antenv lacks.

    bass_utils' trace path imports antenv.axon_hooks; the boot helper that
    can construct the actual hook exists, so wire it up dynamically.
    """
    import types
    try:
        import antenv.axon_hooks  # noqa: F401
        return True
    except ImportError:
        pass
    try:
        import antenv
        from trn_agent_boot.trn_boot import _ntff_profile_via_ctypes
        hook = _ntff_profile_via_ctypes("/opt/axon/libaxon_pjrt.so")
        mod = types.ModuleType("antenv.axon_hooks")
        mod.get_axon_ntff_profile_hook = lambda: hook
        mod.set_axon_ntff_profile_hook = lambda h: None
        sys.modules["antenv.axon_hooks"] = mod
        antenv.axon_hooks = mod
        return hook is not None
    except Exception:
        return False


def kernel(**inputs):
    inputs = {k: np.asarray(v) for k, v in inputs.items()}
    in_maps = _shard_inputs(**inputs)
    if "nc" not in _CACHE:
        _CACHE["nc"] = _build_program()
    nc = _CACHE["nc"]
    trace = bool(os.environ.get("KERNEL_PROFILE")) and _install_profile_shim()
    res = run_bass_kernel_spmd(nc, in_maps, core_ids=list(range(NCORES)),
                               trace=trace)
    LAST_RUN["exec_time_ns"] = res.exec_time_ns
    LAST_RUN["mean_exec_time_ns"] = res.mean_exec_time_ns
    if res.instructions_and_trace is not None:
        LAST_RUN["trace_path"] = res.instructions_and_trace[1]
    return res.results[0]["out"].reshape(1, S, 1)


if __name__ == "__main__":
    rng = np.random.default_rng(0)
    ins = {
        "x": rng.standard_normal((T, S, G), dtype=np.float32),
        "W1": rng.standard_normal((T, G, H), dtype=np.float32) * 0.5,
        "b1": rng.standard_normal((T, G, H), dtype=np.float32) * 0.1,
        "W2": rng.standard_normal((T, G, H), dtype=np.float32) * 0.5,
        "b2": rng.standard_normal((T, G), dtype=np.float32) * 0.1,
        "Wc": rng.standard_normal((G, T), dtype=np.float32) * 0.5,
        "bc": rng.standard_normal((G,), dtype=np.float32) * 0.1,
        "CW0": rng.standard_normal((N1, G), dtype=np.float32) * 0.007,
        "Cb0": rng.standard_normal((N1,), dtype=np.float32) * 0.007,
        "CW1": rng.standard_normal((N2, N1), dtype=np.float32) * 0.02,
        "Cb1": rng.standard_normal((N2,), dtype=np.float32) * 0.02,
        "CW2": rng.standard_normal((N3, N2), dtype=np.float32) * 0.07,
        "Cb2": rng.standard_normal((N3,), dtype=np.float32) * 0.07,
        "CWf": rng.standard_normal((1, N3), dtype=np.float32) * 0.2,
        "Cbf": rng.standard_normal((1,), dtype=np.float32) * 0.2,
    }
    out = kernel(**ins)
    # numpy reference
    xx = ins["x"]
    h = np.maximum(xx[..., None] * ins["W1"][:, None] + ins["b1"][:, None], 0.0)
    y = np.einsum("tsgh,tgh->tsg", h, ins["W2"]) + ins["b2"][:, None, :]
    zz = np.einsum("tsg,gt->sg", y, ins["Wc"]) + ins["bc"]
    for Wl, bl in ((ins["CW0"], ins["Cb0"]), (ins["CW1"], ins["Cb1"]),
                   (ins["CW2"], ins["Cb2"])):
        zz = np.maximum(zz @ Wl.T + bl, 0.0)
    ref = (zz @ ins["CWf"].T + ins["Cbf"])[None]
    err = np.abs(out - ref).max() / (np.abs(ref).max() + 1e-12)
    print("self-test rel err:", err)
    print("exec_time_ns:", LAST_RUN.get("exec_time_ns"))


# revision 25
# speedup vs baseline: 1.0413x; 1.0193x over previous
"""Trainium2 Bass kernel for nn_CombinedAMLModel (dense_mlp, 8 NeuronCores).

Sharding: tensor-parallel over the gene axis (20000 genes -> 2500 per core).

Per core:
  Phase A  - per-(tech,gene) 1->4->1 MLPs plus the per-gene tech combinor,
             computed as 12 relu-affine passes (genes on partitions, per-
             partition scale/bias on ACT/DVE), accumulated into PSUM with
             diagonal fp32r matmuls whose diagonals carry W2[t,g,h]*Wc[g,t].
             The constant term (sum_t b2*Wc + bc) is added during the
             PSUM->SBUF copy. Produces z[g_local, s] (2500 x 1024).
  Phase B  - out1T[n, s] += CW0T[g, n].T @ z[g, s]  (fp32r, K=2500 local
             genes, n=2000), written to DRAM as this core's partial.
  Phase C  - AllReduce of the (2000, 1024) partials across 8 cores.
  Phase D  - tail MLP 2000->200->20->1, replicated on every core, computed
             entirely in transposed orientation (layer outputs on partitions,
             samples on the free axis) so no transposes are needed anywhere.

All matmuls run in float32r (full-rate fp32 matmul, ~1e-4 relative error).
"""
import os
import sys

sys.path.insert(0, "/opt/trn_rl_repo")

import ml_dtypes
import numpy as np
from contextlib import ExitStack

import concourse.bass as bass
import concourse.tile as tile
from concourse import bacc, mybir
from concourse.bass_utils import run_bass_kernel_spmd

T, S, G, H = 3, 1024, 20000, 4
NCORES = 8
GL = G // NCORES            # genes per core
PT = 125                    # gene-tile partition size
NGT = GL // PT              # gene tiles per core
NK = T * H                  # local relu-affine passes
N1, N2, N3 = 2000, 200, 20
PN = 125                    # n-tile partition size for layer-1 output
NNT = N1 // PN              # n tiles
SH = 512                    # PSUM-bank half of the sample axis
ACT_KS = frozenset((0, 2, 4, 6, 8, 10, 11))  # passes on ScalarE; rest on VectorE

f32 = mybir.dt.float32
f32r = mybir.dt.float32r

LAST_RUN = {}
_CACHE = {}


def _build_program():
    nc = bacc.Bacc("TRN2", target_bir_lowering=False, debug=False,
                   num_devices=NCORES)
    d = {}

    def inp(name, shape, dt=f32):
        d[name] = nc.dram_tensor(name, list(shape), dt, kind="ExternalInput").ap()

    inp("xT", (NGT, PT, T * S), mybir.dt.bfloat16)
    inp("scl", (PT, NGT * NK))
    inp("bia", (PT, NGT * NK))
    inp("cst", (PT, NGT))
    inp("ident", (PT, PT))
    inp("coe", (PT, NGT * NK))
    inp("cw0t", (NNT, PT, NGT * PN), mybir.dt.bfloat16)
    inp("cb0", (PN, 2))
    inp("cw1t", (PN, 2 * N2), f32r)
    inp("cb1", (100, 2))
    inp("cw2t", (N2, N3), f32r)
    inp("cb2", (N3, 1))
    inp("cwft", (N3, 1), f32r)
    inp("cbf", (1, 1))
    out_d = nc.dram_tensor("out", [1, S], f32, kind="ExternalOutput").ap()

    Relu = mybir.ActivationFunctionType.Relu
    Ident = mybir.ActivationFunctionType.Identity

    with tile.TileContext(nc) as tc, ExitStack() as ctx:
        const = ctx.enter_context(tc.tile_pool(name="const", bufs=1))
        xpool = ctx.enter_context(tc.tile_pool(name="x", bufs=12))
        dpool = ctx.enter_context(tc.tile_pool(name="diag", bufs=3))
        apool = ctx.enter_context(tc.tile_pool(name="a", bufs=3))
        vpool = ctx.enter_context(tc.tile_pool(name="v", bufs=2))
        zpool = ctx.enter_context(tc.tile_pool(name="z", bufs=NGT))
        wpool = ctx.enter_context(tc.tile_pool(name="w0", bufs=6))
        opool = ctx.enter_context(tc.tile_pool(name="o1", bufs=2))
        tpool = ctx.enter_context(tc.tile_pool(name="tail", bufs=1))
        zps = ctx.enter_context(tc.tile_pool(name="zps", bufs=4, space="PSUM"))
        mmps = ctx.enter_context(tc.tile_pool(name="mmps", bufs=4, space="PSUM"))
        dram = ctx.enter_context(tc.tile_pool(name="dram", bufs=1, space="DRAM"))

        # x preload for the first two gene tiles ahead of everything else
        # (HWDGE drains FIFO per engine; these gate the phase-A ramp).
        x_pre = {}
        for gt in range(2):
            for t in range(T):
                xt = xpool.tile([PT, S], mybir.dt.bfloat16, tag="x",
                                name=f"x{gt}_{t}")
                (nc.gpsimd, nc.sync, nc.gpsimd)[(gt * T + t) % 3].dma_start(
                    xt[:], d["xT"][gt, :, t * S:(t + 1) * S])
                x_pre[(gt, t)] = xt

        sclt = const.tile([PT, NGT * NK], f32)
        nc.scalar.dma_start(sclt[:], d["scl"][:])
        identt = const.tile([PT, PT], f32)
        nc.scalar.dma_start(identt[:], d["ident"][:])
        coet = const.tile([PT, NGT * NK], f32)
        nc.scalar.dma_start(coet[:], d["coe"][:])
        biat = const.tile([PT, NGT * NK], f32)
        nc.scalar.dma_start(biat[:], d["bia"][:])
        cstt = const.tile([PT, NGT], f32)
        nc.scalar.dma_start(cstt[:], d["cst"][:])
        cb0t = const.tile([PN, 2], f32)
        nc.scalar.dma_start(cb0t[:], d["cb0"][:])
        w1t = const.tile([PN, 2 * N2], f32r)
        nc.scalar.dma_start(w1t[:], d["cw1t"][:])
        cb1t = const.tile([100, 2], f32)
        nc.scalar.dma_start(cb1t[:], d["cb1"][:])
        cb2t = const.tile([N3, 1], f32)
        nc.scalar.dma_start(cb2t[:], d["cb2"][:])
        cwftt = const.tile([N3, 1], f32r)
        nc.scalar.dma_start(cwftt[:], d["cwft"][:])
        cbft = const.tile([1, 1], f32)
        nc.scalar.dma_start(cbft[:], d["cbf"][:])
        cw2tt = const.tile([100, 2 * N3], f32r)
        for mc in range(2):
            nc.scalar.dma_start(cw2tt[:, mc * N3:(mc + 1) * N3],
                              d["cw2t"][mc * 100:(mc + 1) * 100, :])

        NCH = 2                       # reduce-scatter chunks (1000 rows each)
        partial_c = [dram.tile([N1 // NCH, S], f32, tag=f"pc{j}",
                               name=f"partial{j}") for j in range(NCH)]
        rs_c = [dram.tile([PN, S], f32, tag=f"rs{j}",
                          name=f"rsout{j}") for j in range(NCH)]
        partial2 = [dram.tile([N2, S], f32, tag=f"p2{j}",
                               name=f"partial2{j}") for j in range(NCH)]
        summed2 = [dram.tile([N2, S], f32, tag=f"s2{j}",
                             name=f"summed2{j}") for j in range(NCH)]
        ccwarm_in = dram.tile([1, 128], f32, tag="ccwi")
        ccwarm_out = dram.tile([1, 128], f32, tag="ccwo")

        # tiny warm-up collective: absorbs the first-rendezvous / ncfw
        # cold-start cost during phase A instead of on the critical tail
        ccwarm_sb = const.tile([1, 128], f32)
        nc.gpsimd.memset(ccwarm_sb[:], 0.0)
        nc.gpsimd.dma_start(ccwarm_in[:], ccwarm_sb[:])
        nc.gpsimd.collective_compute(
            "AllReduce", mybir.AluOpType.add,
            replica_groups=[list(range(NCORES))],
            ins=[ccwarm_in.opt()], outs=[ccwarm_out.opt()],
        )


        # w0 prefetch: first three layer-1 weight blocks load during phase A
        # (HWDGE queues drain FIFO per engine, so these must precede the
        # phase-A loads in program order to be ready when phase B starts).
        HW0 = NGT * PN // 2

        def load_w0(nt):
            w = wpool.tile([PT, NGT * PN], mybir.dt.bfloat16, tag="w0", name=f"w0_{nt}")
            nc.scalar.dma_start(w[:, :HW0], d["cw0t"][nt, :, :HW0])
            nc.gpsimd.dma_start(w[:, HW0:], d["cw0t"][nt, :, HW0:])
            return w

        w_tiles = {nt: load_w0(nt) for nt in range(3)}

        # ---------------- Phase A: local gene MLPs + combinor ----------------
        z_tiles = []
        for gt in range(NGT):
            if gt < 2:
                xts = [x_pre[(gt, t)] for t in range(T)]
            else:
                xts = []
                for t in range(T):
                    xt = xpool.tile([PT, S], mybir.dt.bfloat16, tag="x",
                                    name=f"x{gt}_{t}")
                    (nc.gpsimd, nc.sync, nc.gpsimd)[(gt * T + t) % 3].dma_start(
                        xt[:], d["xT"][gt, :, t * S:(t + 1) * S])
                    xts.append(xt)
            pss = (zps.tile([PT, SH], f32, tag="zps", name=f"zps{gt}_0"),
                   zps.tile([PT, SH], f32, tag="zps", name=f"zps{gt}_1"))
            for k in range(NK):
                t = k // H
                ci = gt * NK + k
                if k in ACT_KS:
                    a = apool.tile([PT, S], f32r, tag="a")
                    nc.scalar.activation(a[:], xts[t], Relu,
                                         bias=biat[:, ci:ci + 1],
                                         scale=sclt[:, ci:ci + 1])
                else:
                    v = vpool.tile([PT, S], f32, tag="v")
                    nc.vector.tensor_scalar(v[:], xts[t],
                                            sclt[:, ci:ci + 1],
                                            biat[:, ci:ci + 1],
                                            mybir.AluOpType.mult,
                                            mybir.AluOpType.add)
                    a = apool.tile([PT, S], f32r, tag="a")
                    nc.vector.tensor_scalar(a[:], v[:], 0.0, None,
                                            mybir.AluOpType.max)
                dg = dpool.tile([PT, PT], f32r, tag="diag", name=f"dg{gt}_{k}")
                nc.vector.tensor_scalar(dg[:], identt[:], coet[:, ci:ci + 1],
                                        None, mybir.AluOpType.mult)
                for sh in range(2):
                    nc.tensor.matmul(pss[sh][:], dg[:],
                                     a[:, sh * SH:(sh + 1) * SH],
                                     start=(k == 0), stop=(k == NK - 1))
            z = zpool.tile([PT, S], mybir.dt.bfloat16, tag="z")
            for sh in range(2):
                nc.scalar.activation(z[:, sh * SH:(sh + 1) * SH], pss[sh][:],
                                     Ident, bias=cstt[:, gt:gt + 1], scale=1.0)
            z_tiles.append(z)

        # ---------------- Phase B: out1T = CW0 @ z (local-gene partial) ------
        # chunked: as each 500-row quarter of the partial is stored, its
        # AllReduce is issued so comm overlaps the remaining matmuls.
        partial_cr = [p[:].rearrange("(g p) s -> p g s", p=PN)
                      for p in partial_c]
        for ng in range(NNT // 2):
            o = opool.tile([PN, 2 * S], f32, tag="o1")
            for j in range(2):
                nt = ng * 2 + j
                if nt in w_tiles:
                    w = w_tiles.pop(nt)
                else:
                    w = load_w0(nt)
                pp = (mmps.tile([PN, SH], f32, tag="mm", name=f"mm{nt}_0"),
                      mmps.tile([PN, SH], f32, tag="mm", name=f"mm{nt}_1"))
                for gt in range(NGT):
                    for sh in range(2):
                        nc.tensor.matmul(pp[sh][:],
                                         w[:, gt * PN:(gt + 1) * PN],
                                         z_tiles[gt][:, sh * SH:(sh + 1) * SH],
                                         start=(gt == 0), stop=(gt == NGT - 1))
                for sh in range(2):
                    nc.scalar.copy(o[:, j * S + sh * SH:j * S + (sh + 1) * SH],
                                   pp[sh][:])
            j, ng2 = divmod(ng, 4)
            nc.sync.dma_start(
                partial_cr[j][:, ng2 * 2:(ng2 + 1) * 2, :], o[:])
            if ng2 == 3:
                nc.gpsimd.collective_compute(
                    "ReduceScatter", mybir.AluOpType.add,
                    replica_groups=[list(range(NCORES))],
                    ins=[partial_c[j].opt()], outs=[rs_c[j].opt()],
                )

        # ------- Phase D: distributed 2000->200 (each core owns 250 rows of
        # the 2000-dim layer via ReduceScatter), then AllReduce the small
        # (200, S) partial and finish 200->20->1 replicated. -------------
        z1_tiles = []
        for j in range(NCH):
            y1 = apool.tile([PN, S], f32, tag="a", name=f"y1_{j}")
            nc.sync.dma_start(y1[:], rs_c[j][:])
            z1 = tpool.tile([PN, S], f32r, tag=f"z1_{j}", name=f"z1_{j}")
            nc.scalar.activation(z1[:], y1[:], Relu,
                                 bias=cb0t[:, j:j + 1], scale=1.0)
            z1_tiles.append(z1)
            for mc in range(2):
                o2 = opool.tile([100, S], f32, tag="o1", name=f"o2_{j}{mc}")
                for sh in range(2):
                    ps = mmps.tile([100, SH], f32, tag="mm",
                                   name=f"ps2_{j}{mc}{sh}")
                    nc.tensor.matmul(
                        ps[:], w1t[:, j * N2 + mc * 100:j * N2 + (mc + 1) * 100],
                        z1[:, sh * SH:(sh + 1) * SH], start=True, stop=True)
                    nc.scalar.copy(o2[:, sh * SH:(sh + 1) * SH], ps[:])
                nc.sync.dma_start(partial2[j][mc * 100:(mc + 1) * 100, :], o2[:])
            nc.gpsimd.collective_compute(
                "AllReduce", mybir.AluOpType.add,
                replica_groups=[list(range(NCORES))],
                ins=[partial2[j].opt()], outs=[summed2[j].opt()],
            )
        z2all = tpool.tile([100, 2 * S], f32r, tag="z2")
        z2_tiles = [z2all[:, 0:S], z2all[:, S:2 * S]]
        for mc in range(2):
            y2a = apool.tile([100, S], f32, tag="a", name=f"y2a_{mc}")
            nc.sync.dma_start(y2a[:], summed2[0][mc * 100:(mc + 1) * 100, :])
            y2b = apool.tile([100, S], f32, tag="a", name=f"y2b_{mc}")
            nc.scalar.dma_start(y2b[:], summed2[1][mc * 100:(mc + 1) * 100, :])
            nc.vector.tensor_tensor(y2a[:], y2a[:], y2b[:],
                                    mybir.AluOpType.add)
            nc.scalar.activation(z2_tiles[mc][:], y2a[:], Relu,
                                 bias=cb1t[:, mc:mc + 1], scale=1.0)
        z3 = tpool.tile([N3, S], f32r, tag="z3")
        for sh in range(2):
            ps = mmps.tile([N3, SH], f32, tag="mm")
            for mc in range(2):
                nc.tensor.matmul(ps[:], cw2tt[:, mc * N3:(mc + 1) * N3],
                                 z2_tiles[mc][:, sh * SH:(sh + 1) * SH],
                                 start=(mc == 0), stop=(mc == 1))
            nc.scalar.activation(z3[:, sh * SH:(sh + 1) * SH], ps[:], Relu,
                                 bias=cb2t[:], scale=1.0)
        outt = tpool.tile([1, S], f32, tag="outt")
        for sh in range(2):
            ps = mmps.tile([1, SH], f32, tag="mm")
            nc.tensor.matmul(ps[:], cwftt[:],
                             z3[:, sh * SH:(sh + 1) * SH],
                             start=True, stop=True)
            nc.scalar.activation(outt[:, sh * SH:(sh + 1) * SH], ps[:], Ident,
                                 bias=cbft[:], scale=1.0)
        nc.sync.dma_start(out_d[:], outt[:])

    nc.compile()
    return nc


def _shard_inputs(x, W1, b1, W2, b2, Wc, bc,
                  CW0, Cb0, CW1, Cb1, CW2, Cb2, CWf, Cbf):
    f = lambda a: np.ascontiguousarray(a, dtype=np.float32)
    CW1T = np.ascontiguousarray(CW1.T)
    shared = {
        "cb1": f(Cb1.reshape(2, 100).T),
        "cw2t": f(CW2.T),
        "cb2": f(Cb2.reshape(N3, 1)),
        "cwft": f(CWf.T),
        "cbf": f(Cbf.reshape(1, 1)),
    }
    in_maps = []
    for c in range(NCORES):
        gs = slice(c * GL, (c + 1) * GL)
        scl = W1[:, gs, :].transpose(1, 0, 2).reshape(GL, NK)
        bia = b1[:, gs, :].transpose(1, 0, 2).reshape(GL, NK)
        coe = (W2[:, gs, :] * Wc[gs, :].T[:, :, None]) \
            .transpose(1, 0, 2).reshape(GL, NK)
        cst = (b2[:, gs] * Wc[gs, :].T).sum(0) + bc[gs]
        half = N1 // 2
        in_maps.append({
            "cb0": f(np.stack([Cb0[c * PN:(c + 1) * PN],
                               Cb0[half + c * PN:half + (c + 1) * PN]], axis=1)),
            "cw1t": f(np.concatenate(
                [CW1T[c * PN:(c + 1) * PN, :],
                 CW1T[half + c * PN:half + (c + 1) * PN, :]], axis=1)),
            "xT": np.ascontiguousarray(
                x[:, :, gs].transpose(2, 0, 1).reshape(NGT, PT, T * S)
            ).astype(ml_dtypes.bfloat16),
            "scl": f(scl.reshape(NGT, PT, NK).transpose(1, 0, 2)
                     .reshape(PT, NGT * NK)),
            "bia": f(bia.reshape(NGT, PT, NK).transpose(1, 0, 2)
                     .reshape(PT, NGT * NK)),
            "cst": f(cst.reshape(NGT, PT).T),
            "ident": np.eye(PT, dtype=np.float32),
            "coe": f(coe.reshape(NGT, PT, NK).transpose(1, 0, 2)
                     .reshape(PT, NGT * NK)),
            "cw0t": np.ascontiguousarray(
                CW0[:, gs].reshape(NNT, PN, NGT, PT)
                .transpose(0, 3, 2, 1).reshape(NNT, PT, NGT * PN)
            ).astype(ml_dtypes.bfloat16),
            **shared,
        })
    return in_maps


def _install_profile_shim():
    """Register the NTFF profiling hook that this container's antenv lacks.

    bass_utils' trace path imports antenv.axon_hooks; the boot helper that
    can construct the actual hook exists, so wire it up dynamically.
    """
    import types
    try:
        import antenv.axon_hooks  # noqa: F401
        return True
    except ImportError:
        pass
    try:
        import antenv
        from trn_agent_boot.trn_boot import _ntff_profile_via_ctypes
        hook = _ntff_profile_via_ctypes("/opt/axon/libaxon_pjrt.so")
        mod = types.ModuleType("antenv.axon_hooks")
        mod.get_axon_ntff_profile_hook = lambda: hook
        mod.set_axon_ntff_profile_hook = lambda h: None
        sys.modules["antenv.axon_hooks"] = mod
        antenv.axon_hooks = mod
        return hook is not None
    except Exception:
        return False


def kernel(**inputs):
    inputs = {k: np.asarray(v) for k, v in inputs.items()}
    in_maps = _shard_inputs(**inputs)
    if "nc" not in _CACHE:
        _CACHE["nc"] = _build_program()
    nc = _CACHE["nc"]
    trace = bool(os.environ.get("KERNEL_PROFILE")) and _install_profile_shim()
    res = run_bass_kernel_spmd(nc, in_maps, core_ids=list(range(NCORES)),
                               trace=trace)
    LAST_RUN["exec_time_ns"] = res.exec_time_ns
    LAST_RUN["mean_exec_time_ns"] = res.mean_exec_time_ns
    if res.instructions_and_trace is not None:
        LAST_RUN["trace_path"] = res.instructions_and_trace[1]
    return res.results[0]["out"].reshape(1, S, 1)


if __name__ == "__main__":
    rng = np.random.default_rng(0)
    ins = {
        "x": rng.standard_normal((T, S, G), dtype=np.float32),
        "W1": rng.standard_normal((T, G, H), dtype=np.float32) * 0.5,
        "b1": rng.standard_normal((T, G, H), dtype=np.float32) * 0.1,
        "W2": rng.standard_normal((T, G, H), dtype=np.float32) * 0.5,
        "b2": rng.standard_normal((T, G), dtype=np.float32) * 0.1,
        "Wc": rng.standard_normal((G, T), dtype=np.float32) * 0.5,
        "bc": rng.standard_normal((G,), dtype=np.float32) * 0.1,
        "CW0": rng.standard_normal((N1, G), dtype=np.float32) * 0.007,
        "Cb0": rng.standard_normal((N1,), dtype=np.float32) * 0.007,
        "CW1": rng.standard_normal((N2, N1), dtype=np.float32) * 0.02,
        "Cb1": rng.standard_normal((N2,), dtype=np.float32) * 0.02,
        "CW2": rng.standard_normal((N3, N2), dtype=np.float32) * 0.07,
        "Cb2": rng.standard_normal((N3,), dtype=np.float32) * 0.07,
        "CWf": rng.standard_normal((1, N3), dtype=np.float32) * 0.2,
        "Cbf": rng.standard_normal((1,), dtype=np.float32) * 0.2,
    }
    out = kernel(**ins)
    # numpy reference
    xx = ins["x"]
    h = np.maximum(xx[..., None] * ins["W1"][:, None] + ins["b1"][:, None], 0.0)
    y = np.einsum("tsgh,tgh->tsg", h, ins["W2"]) + ins["b2"][:, None, :]
    zz = np.einsum("tsg,gt->sg", y, ins["Wc"]) + ins["bc"]
    for Wl, bl in ((ins["CW0"], ins["Cb0"]), (ins["CW1"], ins["Cb1"]),
                   (ins["CW2"], ins["Cb2"])):
        zz = np.maximum(zz @ Wl.T + bl, 0.0)
    ref = (zz @ ins["CWf"].T + ins["Cbf"])[None]
    err = np.abs(out - ref).max() / (np.abs(ref).max() + 1e-12)
    print("self-test rel err:", err)
    print("exec_time_ns:", LAST_RUN.get("exec_time_ns"))
